# revision 16
# baseline (speedup 1.0000x reference)
"""Cross-attention block kernel for Trainium2 (Bass/Tile), SPMD over 8 cores.

Sharding: data-parallel over batch B=8 -> one batch element per NeuronCore.
Per core:
  xn  = LayerNorm(xt) * w + b                      [4096, 128]
  cn  = LayerNorm(context) * cw + cb               [256, 768]
  q   = xn @ Wq                                    [4096, 512]  (8 heads x 64)
  k,v = cn @ Wkv (+ null kv row)                   [257, 512] each
  sim = q @ k^T / 8, softmax over keys (mask folded into v-side),
  out = attn @ v ; final = out @ Wout + bout + xn  [4096, 128]

Layout strategy (keys on partitions for the softmax/AV stage):
  xnT  [C=128, 4096]           (PE transposes of LN'd tiles)
  qT   [HID -> 4x128, T]       per 512-token chunk, heads at partition (h%2)*64
  kT   [D=64, 256] per head    packed 2 heads/tile at partitions 0/64
  simT [keys, T] in PSUM, p = exp(simT/8) (no max subtraction needed:
       |sim/8| <~ 2, and masked keys are zeroed via the v_aug rows)
  v_aug[keys, 65] = [v*mask, mask] -> one matmul pair gives out rows 0:64 and
       the ctx-key denominator in row 64.
  The null key is folded away: null_sim comes free as 8 extra Wq columns
  (Wq_h @ k_null), and the null value's contribution enters the final
  projection as one K=8 matmul with Vnull' = Wout_h^T @ v_null.
  finalT = Wout^T @ outT (+bout +xnT residual), PE-transposed back to [tok, C].
"""

import numpy as np

import concourse.bacc as bacc
import concourse.bass as bass
import concourse.mybir as mybir
import concourse.tile as tile
from concourse.bass_utils import run_bass_kernel_spmd
from concourse.masks import make_identity

B, XS, YS, C = 8, 64, 64, 128
CTX, N, H, D = 768, 256, 8, 64
HID = H * D          # 512
TOK = XS * YS        # 4096 tokens per batch element
TCH = 512            # tokens per chunk (PSUM bank free size in fp32)
NT = TOK // TCH      # 8 token chunks
NCORES = 8
F32 = mybir.dt.float32
F32R = mybir.dt.float32r
EPS = 1e-5
SCALE = D ** -0.5
Exp = mybir.ActivationFunctionType.Exp
Sqrt = mybir.ActivationFunctionType.Sqrt
Ident = mybir.ActivationFunctionType.Identity


def build(n_iters: int = 1):
    nc = bacc.Bacc("TRN2", target_bir_lowering=False, debug=False,
                   num_devices=NCORES)

    xt_d = nc.dram_tensor("xt", [TOK, C], F32, kind="ExternalInput")
    ctx_d = nc.dram_tensor("context", [N, CTX], F32, kind="ExternalInput")
    mask_d = nc.dram_tensor("mask", [N], mybir.dt.uint8, kind="ExternalInput")
    nw_d = nc.dram_tensor("norm_w", [C], F32, kind="ExternalInput")
    nb_d = nc.dram_tensor("norm_b", [C], F32, kind="ExternalInput")
    cw_d = nc.dram_tensor("ctx_norm_w", [CTX], F32, kind="ExternalInput")
    cb_d = nc.dram_tensor("ctx_norm_b", [CTX], F32, kind="ExternalInput")
    wq_d = nc.dram_tensor("Wq", [C, HID], F32, kind="ExternalInput")
    wkv_d = nc.dram_tensor("Wkv", [CTX, 2 * HID], F32, kind="ExternalInput")
    nkv_d = nc.dram_tensor("null_kv", [2, D], F32, kind="ExternalInput")
    wout_d = nc.dram_tensor("Wout", [HID, C], F32, kind="ExternalInput")
    bout_d = nc.dram_tensor("bout", [C], F32, kind="ExternalInput")
    out_d = nc.dram_tensor("out", [TOK, C], F32, kind="ExternalOutput")

    def bc_ap(handle, n_part, n_free):
        # broadcast a [n_free] DRAM vector across n_part partitions
        return bass.AP(handle, 0, [[0, n_part], [1, n_free]])

    def col_ap(handle, n_part, row=0):
        # load a [n_free] DRAM vector into n_part partitions x 1
        return bass.AP(handle, row * n_part, [[1, n_part], [1, 1]])

    with tile.TileContext(nc) as tc:
        with (
            tc.tile_pool(name="const", bufs=1) as const,
            tc.tile_pool(name="wides", bufs=1) as wides,
            tc.tile_pool(name="work", bufs=3) as work,
            tc.tile_pool(name="work2", bufs=2) as work2,
            tc.tile_pool(name="persist", bufs=1) as persist,
            tc.tile_pool(name="small", bufs=4) as small,
            # PSUM budget (8 banks): ps 2x2 + po 2 + aux 2
            tc.tile_pool(name="pbig", bufs=2, space=bass.MemorySpace.PSUM) as pbig,
            tc.tile_pool(name="pout", bufs=2, space=bass.MemorySpace.PSUM) as pout,
            tc.tile_pool(name="paux", bufs=2, space=bass.MemorySpace.PSUM) as paux,
        ):
            ident = const.tile([128, 128], F32)
            make_identity(nc, ident)
            identr = const.tile([128, 128], F32R)
            nc.vector.tensor_copy(out=identr, in_=ident)
            eps_t = const.tile([128, 1], F32)
            nc.vector.memset(eps_t, EPS)

            for _ in range(n_iters):
                # ---- weights (gpsimd casting DMAs round fp32 -> f32r) ----
                wq_sb = wides.tile([C, HID], F32R, tag="wq")
                nc.gpsimd.dma_start(out=wq_sb, in_=wq_d.ap())
                wkv_sb = wides.tile([128, 6, 2 * HID], F32R, tag="wkv")
                nc.gpsimd.dma_start(
                    out=wkv_sb,
                    in_=bass.AP(wkv_d, 0, [[2 * HID, 128], [128 * 2 * HID, 6],
                                           [1, 2 * HID]]))
                # Wout as [d=64, head, C] so K=64 matmuls start at partition 0
                wout_sb = wides.tile([D, H, C], F32R, tag="wout")
                nc.gpsimd.dma_start(
                    out=wout_sb,
                    in_=bass.AP(wout_d, 0, [[C, D], [D * C, H], [1, C]]))
                nw_bc = wides.tile([128, C], F32, tag="nw")
                nc.sync.dma_start(out=nw_bc, in_=bc_ap(nw_d, 128, C))
                nb_bc = wides.tile([128, C], F32, tag="nb")
                nc.sync.dma_start(out=nb_bc, in_=bc_ap(nb_d, 128, C))
                cw_bc = wides.tile([128, CTX], F32, tag="cw")
                nc.sync.dma_start(out=cw_bc, in_=bc_ap(cw_d, 128, CTX))
                cb_bc = wides.tile([128, CTX], F32, tag="cb")
                nc.sync.dma_start(out=cb_bc, in_=bc_ap(cb_d, 128, CTX))
                bout_sb = wides.tile([C, 1], F32, tag="bout")
                nc.sync.dma_start(out=bout_sb, in_=col_ap(bout_d, C))

                mask8 = small.tile([128, 2], mybir.dt.uint8, tag="m8")
                nc.sync.dma_start(out=mask8,
                                  in_=bass.AP(mask_d, 0, [[1, 128], [128, 2]]))
                maskf = wides.tile([128, 2], F32, tag="mf")
                nc.vector.tensor_copy(out=maskf, in_=mask8)

                # ---- Wq^T (for null-q columns) ----
                wqT = wides.tile([128, 4, 128], F32R, tag="wqT")
                for m in range(4):
                    ptw = paux.tile([128, 128], F32R, tag="aux")
                    nc.tensor.transpose(ptw, wq_sb[:, m * 128:(m + 1) * 128],
                                        identr)
                    nc.scalar.copy(out=wqT[:, m, :], in_=ptw)
                # wqnull[:, h] = Wq_h @ k_null  (8 extra q columns), via a
                # block-diagonal k_null matrix: wqnullT = kblk^T @ WqT
                kblk = wides.tile([128, 4, H], F32, tag="kblk")
                nc.vector.memset(kblk, 0.0)
                for h in range(H):
                    hb = (h % 2) * 64
                    nc.sync.dma_start(
                        out=kblk[hb:hb + 64, h // 2, h:h + 1],
                        in_=col_ap(nkv_d, D, 0))
                kblk_r = wides.tile([128, 4, H], F32R, tag="kblk_r")
                nc.vector.tensor_copy(out=kblk_r, in_=kblk)
                pwn = paux.tile([H, 128], F32, tag="aux")
                for m in range(4):
                    nc.tensor.matmul(pwn, kblk_r[:, m, :], wqT[:, m, :],
                                     start=(m == 0), stop=(m == 3))
                wqnT = small.tile([H, C], F32, tag="wqnT")
                nc.scalar.copy(out=wqnT, in_=pwn)
                pwt = paux.tile([128, 128], F32, tag="aux")
                nc.tensor.matmul(pwt, wqnT, ident[0:H, :],
                                 is_transpose=True, start=True, stop=True)
                wqnull = wides.tile([C, H], F32R, tag="wqnull")
                nc.scalar.copy(out=wqnull, in_=pwt[:, 0:H])

                # null-value row for the AV matmul: [1, 65] = [v_null, 1]
                vnull = small.tile([1, D + 1], F32, tag="vnull")
                nc.sync.dma_start(out=vnull[:, 0:D],
                                  in_=bass.AP(nkv_d, D, [[1, 1], [1, D]]))
                nc.vector.memset(vnull[:, D:D + 1], 1.0)
                vnull_r = small.tile([1, D + 1], F32R, tag="vnull_r")
                nc.vector.tensor_copy(out=vnull_r, in_=vnull)

                # ---- context LN -> cn, then cnT [768, 256] ----
                cn = []
                for t in range(2):
                    ct = work.tile([128, CTX], F32, tag="cn")
                    nc.sync.dma_start(out=ct, in_=ctx_d.ap()[t * 128:(t + 1) * 128, :])
                    st = small.tile([128, 3, 6], F32, tag="cstat")
                    for g in range(3):
                        nc.vector.bn_stats(out=st[:, g, :], in_=ct[:, g * 256:(g + 1) * 256])
                    mv = small.tile([128, 2], F32, tag="cmv")
                    nc.vector.bn_aggr(out=mv, in_=st)
                    sd = small.tile([128, 1], F32, tag="csd")
                    nc.scalar.activation(out=sd, in_=mv[:, 1:2], func=Sqrt,
                                         bias=eps_t)
                    nc.vector.reciprocal(out=sd, in_=sd)
                    nc.vector.tensor_scalar(out=ct, in0=ct, scalar1=mv[:, 0:1],
                                            scalar2=sd, op0=mybir.AluOpType.subtract,
                                            op1=mybir.AluOpType.mult)
                    nc.gpsimd.tensor_mul(out=ct, in0=ct, in1=cw_bc)
                    nc.gpsimd.tensor_add(out=ct, in0=ct, in1=cb_bc)
                    cn.append(ct)
                cnT = wides.tile([128, 6, N], F32R, tag="cnT")
                for rr0 in range(0, 6, 2):
                    pt4 = paux.tile([128, 4, 128], F32, tag="aux")
                    for j in range(2):
                        for t in range(2):
                            nc.tensor.transpose(
                                pt4[:, 2 * j + t, :],
                                cn[t][:, (rr0 + j) * 128:(rr0 + j + 1) * 128],
                                ident)
                    nc.scalar.copy(out=cnT[:, rr0:rr0 + 2, :], in_=pt4)

                # ---- k,v = cn @ Wkv in [keys, HID] layout (aligned lhsT),
                # then kT per head via PE transposes; v_aug padded to stride 128
                kt_all = wides.tile([128, 4, N], F32R, tag="kt_all")
                va = wides.tile([128, 2, H, D + 1], F32, tag="va")
                for kc in range(2):
                    pkv = paux.tile([128, HID], F32, tag="aux")
                    for rr in range(6):
                        nc.tensor.matmul(
                            pkv, cnT[:, rr, kc * 128:(kc + 1) * 128],
                            wkv_sb[:, rr, 0:HID],
                            start=(rr == 0), stop=(rr == 5))
                    ksb = work.tile([128, HID], F32, tag="ksb")
                    nc.scalar.copy(out=ksb, in_=pkv)
                    ptk4 = paux.tile([128, 4, 128], F32, tag="aux")
                    for c in range(4):
                        nc.tensor.transpose(ptk4[:, c, :],
                                            ksb[:, c * 128:(c + 1) * 128], ident)
                    nc.scalar.copy(out=kt_all[:, :, kc * 128:(kc + 1) * 128],
                                   in_=ptk4)
                    pv = paux.tile([128, HID], F32, tag="aux")
                    for rr in range(6):
                        nc.tensor.matmul(
                            pv, cnT[:, rr, kc * 128:(kc + 1) * 128],
                            wkv_sb[:, rr, HID:2 * HID],
                            start=(rr == 0), stop=(rr == 5))
                    nc.vector.tensor_copy(
                        out=va[:, kc, :, 0:D],
                        in_=pv.rearrange("p (h d) -> p h d", h=H))
                nc.vector.memset(va[:, :, :, D:D + 1], 1.0)
                for kc in range(2):
                    nc.gpsimd.tensor_scalar_mul(va[:, kc], va[:, kc],
                                                maskf[:, kc:kc + 1])
                # rounded copy, padded so each head's lhsT slice is 128-aligned
                va_r = wides.tile([128, 2, H, 128], F32R, tag="va_r")
                nc.vector.tensor_copy(out=va_r[:, :, :, 0:D + 1], in_=va)

                # ---- xt LN -> xn tiles -> xnT [C, 4096] ----
                xnT = wides.tile([C, TOK], F32R, tag="xnT")
                for t in range(TOK // 128):
                    xtile = work.tile([128, C], F32, tag="xt")
                    nc.sync.dma_start(out=xtile,
                                      in_=xt_d.ap()[t * 128:(t + 1) * 128, :])
                    st = small.tile([128, 6], F32, tag="xstat")
                    nc.vector.bn_stats(out=st, in_=xtile)
                    mv = small.tile([128, 2], F32, tag="xmv")
                    nc.vector.bn_aggr(out=mv, in_=st)
                    sd = small.tile([128, 1], F32, tag="xsd")
                    nc.scalar.activation(out=sd, in_=mv[:, 1:2], func=Sqrt,
                                         bias=eps_t)
                    nc.vector.reciprocal(out=sd, in_=sd)
                    nc.vector.tensor_scalar(out=xtile, in0=xtile, scalar1=mv[:, 0:1],
                                            scalar2=sd, op0=mybir.AluOpType.subtract,
                                            op1=mybir.AluOpType.mult)
                    nc.gpsimd.tensor_mul(out=xtile, in0=xtile, in1=nw_bc)
                    nc.gpsimd.tensor_add(out=xtile, in0=xtile, in1=nb_bc)
                    if t % 4 == 0:
                        pt4 = paux.tile([128, 4, 128], F32, tag="aux")
                        pt4_live = pt4
                    nc.tensor.transpose(pt4_live[:, t % 4, :], xtile, ident)
                    if t % 4 == 3:
                        nc.scalar.copy(
                            out=xnT[:, (t - 3) * 128:(t + 1) * 128], in_=pt4_live)

                # ---- attention per 512-token chunk ----
                for t in range(NT):
                    tsl = slice(t * TCH, (t + 1) * TCH)
                    qT = work2.tile([128, 4, TCH], F32R, tag="qT")
                    for m in range(4):
                        pq = paux.tile([128, TCH], F32, tag="aux")
                        nc.tensor.matmul(pq, wq_sb[:, m * 128:(m + 1) * 128],
                                         xnT[:, tsl], start=True, stop=True)
                        if m % 2 == 0:
                            nc.scalar.copy(out=qT[:, m, :], in_=pq)
                        else:
                            nc.vector.tensor_copy(out=qT[:, m, :], in_=pq)
                    # null-key sims for all 8 heads at once
                    pq5 = paux.tile([128, TCH], F32, tag="aux")
                    nc.tensor.matmul(pq5[0:H, :], wqnull, xnT[:, tsl],
                                     start=True, stop=True)
                    pn_exp = small.tile([H, TCH], F32R, tag="pn_exp")
                    nc.scalar.activation(out=pn_exp, in_=pq5[0:H, :], func=Exp,
                                         scale=SCALE)
                    # rearrange to one partition so each head's row is a
                    # base-0 matmul rhs
                    pn1 = persist.tile([1, H, TCH], F32R, tag="pn1")
                    nc.sync.dma_start(out=pn1, in_=pn_exp)
                    outT = persist.tile([D, H, TCH], F32R, tag="outT")
                    for h in range(H):
                        hb = (h % 2) * 64
                        qh = qT[hb:hb + 64, h // 2, :]
                        ps = pbig.tile([128, 2, TCH], F32, tag="ps")
                        nc.tensor.matmul(ps[:, 0, :],
                                         kt_all[hb:hb + 64, h // 2, 0:128],
                                         qh, start=True, stop=True)
                        nc.tensor.matmul(ps[:, 1, :],
                                         kt_all[hb:hb + 64, h // 2, 128:256],
                                         qh, start=True, stop=True)
                        pe = work.tile([128, 2, TCH], F32R, tag="pexp")
                        nc.scalar.activation(out=pe, in_=ps, func=Exp,
                                             scale=SCALE)
                        po = pout.tile([D + 1, TCH], F32, tag="po")
                        nc.tensor.matmul(po, va_r[:, 0, h, 0:D + 1],
                                         pe[:, 0, :], start=True, stop=False)
                        nc.tensor.matmul(po, va_r[:, 1, h, 0:D + 1],
                                         pe[:, 1, :], start=False, stop=False)
                        nc.tensor.matmul(po, vnull_r, pn1[:, h, :],
                                         start=False, stop=True)
                        rc = small.tile([1, TCH], F32, tag="rc")
                        nc.vector.reciprocal(out=rc, in_=po[D:D + 1, :])
                        rb = work.tile([D, TCH], F32, tag="rb")
                        nc.gpsimd.partition_broadcast(rb, rc)
                        nc.vector.tensor_mul(out=outT[:, h, :],
                                             in0=po[0:D, :], in1=rb)

                    # ---- final projection + bias + residual ----
                    pf = paux.tile([C, TCH], F32, tag="aux")
                    for h in range(H):
                        nc.tensor.matmul(pf, wout_sb[:, h, :], outT[:, h, :],
                                         start=(h == 0), stop=(h == H - 1))
                    fT = work.tile([C, TCH], F32, tag="fT")
                    nc.scalar.activation(out=fT, in_=pf, func=Ident,
                                         bias=bout_sb)
                    nc.gpsimd.tensor_add(out=fT, in0=fT,
                                         in1=xnT[:, tsl].bitcast(F32))
                    pt4 = paux.tile([128, 4, 128], F32, tag="aux")
                    for sblk in range(4):
                        nc.tensor.transpose(pt4[:, sblk, :],
                                            fT[:, sblk * 128:(sblk + 1) * 128],
                                            ident)
                    fo = work.tile([128, 4, C], F32, tag="fo")
                    nc.scalar.copy(out=fo, in_=pt4)
                    orows = out_d.ap()[t * TCH:(t + 1) * TCH, :]
                    nc.sync.dma_start(
                        out=orows.rearrange("(s p) c -> p s c", p=128), in_=fo)

    nc.compile()
    return nc


_CACHE = {}


def get_nc(n_iters: int = 1):
    if n_iters not in _CACHE:
        _CACHE[n_iters] = build(n_iters)
    return _CACHE[n_iters]


def make_in_maps(xt, context, mask, norm_w, norm_b, ctx_norm_w, ctx_norm_b,
                 Wq, Wkv, null_kv, Wout, bout):
    xt = np.asarray(xt, dtype=np.float32).reshape(B, TOK, C)
    context = np.asarray(context, dtype=np.float32)
    mask8 = np.asarray(mask).astype(np.uint8)
    shared = {
        "norm_w": np.asarray(norm_w, np.float32),
        "norm_b": np.asarray(norm_b, np.float32),
        "ctx_norm_w": np.asarray(ctx_norm_w, np.float32),
        "ctx_norm_b": np.asarray(ctx_norm_b, np.float32),
        "Wq": np.asarray(Wq, np.float32),
        "Wkv": np.asarray(Wkv, np.float32),
        "null_kv": np.asarray(null_kv, np.float32),
        "Wout": np.asarray(Wout, np.float32),
        "bout": np.asarray(bout, np.float32),
    }
    return [
        {"xt": xt[b], "context": context[b], "mask": mask8[b], **shared}
        for b in range(B)
    ]


def kernel(xt, context, mask, norm_w, norm_b, ctx_norm_w, ctx_norm_b,
           Wq, Wkv, null_kv, Wout, bout):
    nc = get_nc(1)
    in_maps = make_in_maps(xt, context, mask, norm_w, norm_b, ctx_norm_w,
                           ctx_norm_b, Wq, Wkv, null_kv, Wout, bout)
    res = run_bass_kernel_spmd(nc, in_maps, core_ids=list(range(NCORES)))
    out = np.stack([res.results[b]["out"] for b in range(B)], axis=0)
    return out.reshape(B, XS, YS, C).astype(np.float32)
